# revision 5
# baseline (speedup 1.0000x reference)
"""DifferentiableMatcher Trainium2 kernel (v3).

cost[k, n] = 1 - <pred_k, gt_n> over HW=512*512, then 5 Sinkhorn iterations
(row/col normalizations) and exp.

Strategy (8 NeuronCores):
  - Shard the HW contraction: core c owns HW slice [c*32768, (c+1)*32768).
  - Inputs cast to fp16 on host (halves HBM traffic; fp32 PSUM accumulate).
  - Host packs each shard so SBUF partition p holds runs of FB=4 HW elements
    per (q, k); DMA per partition is fully contiguous.
  - Per core: 256 accumulating fp16 matmuls -> partial dot [100, 50] in PSUM.
    Block sizes ramp 1,1,2,4,8,...,8,4,2,1,1 q-steps so the PE starts on the
    first small block almost immediately and drains right as the DMA stream
    ends (PE period ~104ns/matmul ~= DMA rate).
  - AllReduce (20KB fp32) across the 8 cores.
  - Sinkhorn v3: iteration 1 runs in log space (row lse with max; col step
    computes exp(x - max)/sum directly), then iterations 2-5 run in EXP
    space: divide by row/col sums only.  Each transpose is a plain fp32
    matmul against [I | 1] which also emits the next step's sums column,
    so a half-step is just (tensor_scalar divide) + (matmul).  The final
    col division is the output (exp already applied by construction).

v3 changes vs v2:
  - DMA block ramp (PE drain overlapped with stream end).
  - Exp-space Sinkhorn: 2 instructions per half-step instead of 6.
"""

import numpy as np

K = 100
N = 50
HW = 512 * 512
CORES = 8
SHARD = HW // CORES  # 32768
P = 128
FB = 4
Q = SHARD // (P * FB)  # 64 q-steps per core
# q-steps per block: small ramp-in so the PE starts early, small ramp-out so
# the last matmuls aren't serialized behind one big DMA.
BLOCKS = [1, 2, 4, 8, 8, 8, 8, 8, 8, 8, 1]
assert sum(BLOCKS) == Q
SIZES = sorted(set(BLOCKS))  # [1, 2, 4, 8]
TEMP = 0.1
ITERS = 5

_CACHE = {}

TRACE = False
TRACE_KW = {}
LAST_RESULT = None


def _patch_act_tables():
    """Make the combined Exp+Ln table set the only candidate for Exp/Ln so
    the compiler emits one table load instead of thrashing per activation.
    Set positions (= act_func_set_ids) are preserved."""
    import concourse.hw_specs as hw_specs
    from concourse import bacc as bacc_mod
    from concourse import mybir

    if getattr(bacc_mod, "_act_tables_patched", False):
        return
    orig = hw_specs.get_activation_tables

    def patched(arch):
        t = orig(arch)
        exp = mybir.ActivationFunctionType.Exp
        ln = mybir.ActivationFunctionType.Ln
        out = {}
        for name, funcs in t.items():
            if (exp in funcs) != (ln in funcs):
                funcs = funcs - {exp, ln}
            out[name] = funcs
        return out

    bacc_mod.get_activation_tables = patched
    bacc_mod._act_tables_patched = True


def _build():
    from concourse import bacc, tile, mybir
    from concourse.masks import make_identity

    _patch_act_tables()

    f16 = mybir.dt.float16
    f32 = mybir.dt.float32
    nc = bacc.Bacc("TRN2", target_bir_lowering=False, debug=False, enable_asserts=False, num_devices=CORES, monotonic_sem_count=0, enable_partition_id=False)

    counts = {s: BLOCKS.count(s) for s in SIZES}
    p_in = {
        s: nc.dram_tensor(f"p_{s}", [counts[s], P, s * K * FB], f16, kind="ExternalInput").ap()
        for s in SIZES
    }
    g_in = {
        s: nc.dram_tensor(f"g_{s}", [counts[s], P, s * N * FB], f16, kind="ExternalInput").ap()
        for s in SIZES
    }
    out = nc.dram_tensor("out", [N, K], f32, kind="ExternalOutput").ap()

    Exp = mybir.ActivationFunctionType.Exp
    Ln = mybir.ActivationFunctionType.Ln

    with tile.TileContext(nc) as tc:
        with (
            tc.tile_pool(name="pp", bufs=len(BLOCKS)) as pp,
            tc.tile_pool(name="gp", bufs=len(BLOCKS)) as gp,
            tc.tile_pool(name="sk", bufs=1) as sk,
            tc.tile_pool(name="cps", bufs=1, space="PSUM") as cps,
            tc.tile_pool(name="tps", bufs=2, space="PSUM") as tps,
            tc.tile_pool(name="dram", bufs=1, space="DRAM") as dram,
        ):
            # Force the natural_log_exp table set to load now, while the
            # DMAs stream; the Sinkhorn tail then starts without the
            # 1.3us ACT_TABLE_LOAD on its critical path.
            warm = sk.tile([1, 1], f32)
            nc.vector.memset(warm, 0.0)
            nc.scalar.activation(out=warm, in_=warm, func=Exp)

            # Tiny AllGather fired at kernel start: absorbs the CC-core
            # per-collective setup latency while the input DMAs stream, so
            # the real AllReduce later starts its mesh phase sooner.
            win = dram.tile([1, 1], f32)
            wout = dram.tile([CORES, 1], f32, addr_space="Shared")
            nc.sync.dma_start(out=win, in_=warm)
            nc.gpsimd.collective_compute(
                "AllGather",
                mybir.AluOpType.bypass,
                replica_groups=[list(range(CORES))],
                ins=[win.opt()],
                outs=[wout.opt()],
            )

            # [I_K | 1] and [I_N | 1]: transpose-with-sums right operands.
            # AUGK[:, 0:K] doubles as the identity for the iter-1 transpose.
            augk = sk.tile([K, K + 1], f32)
            make_identity(nc, augk[:, 0:K])
            nc.gpsimd.memset(augk[:, K : K + 1], 1.0)
            augn = sk.tile([N, N + 1], f32)
            make_identity(nc, augn[:, 0:N])
            nc.gpsimd.memset(augn[:, N : N + 1], 1.0)

            C = cps.tile([K, N], f32)
            seen = {s: 0 for s in SIZES}
            nblk = len(BLOCKS)
            for bi, s in enumerate(BLOCKS):
                b = seen[s]
                seen[s] += 1
                PT = pp.tile([P, s * K * FB], f16)
                GT = gp.tile([P, s * N * FB], f16)
                qs = [nc.sync, nc.scalar, nc.gpsimd]
                qg = qs[(2 * bi) % 3]
                qp = qs[(2 * bi + 1) % 3]
                qg.dma_start(out=GT, in_=g_in[s][b])
                qp.dma_start(out=PT, in_=p_in[s][b])
                PT4 = PT.rearrange("p (q k f) -> p q k f", k=K, f=FB)
                GT4 = GT.rearrange("p (q n f) -> p q n f", n=N, f=FB)
                for q in range(s):
                    for f in range(FB):
                        nc.tensor.matmul(
                            C,
                            PT4[:, q, :, f],
                            GT4[:, q, :, f],
                            start=(bi == 0 and q == 0 and f == 0),
                            stop=(bi == nblk - 1 and q == s - 1 and f == FB - 1),
                        )

            # partial dot [K,N] -> (scale 1/TEMP) -> SBUF -> DRAM ->
            # AllReduce -> SBUF
            c_sb = sk.tile([K, N], f32)
            din = dram.tile([K, N], f32)
            dout = dram.tile([K, N], f32, addr_space="Shared")
            nc.vector.tensor_scalar(
                out=c_sb[0:64, :], in0=C[0:64, :], scalar1=1.0 / TEMP,
                scalar2=None, op0=mybir.AluOpType.mult,
            )
            nc.sync.dma_start(out=din[0:64, :], in_=c_sb[0:64, :])
            nc.vector.tensor_scalar(
                out=c_sb[64:K, :], in0=C[64:K, :], scalar1=1.0 / TEMP,
                scalar2=None, op0=mybir.AluOpType.mult,
            )
            nc.scalar.dma_start(out=din[64:K, :], in_=c_sb[64:K, :])
            nc.gpsimd.collective_compute(
                "AllReduce",
                mybir.AluOpType.add,
                replica_groups=[list(range(CORES))],
                ins=[din.opt()],
                outs=[dout.opt()],
            )
            csum = sk.tile([K, N], f32)
            nc.sync.dma_start(out=csum[0:64, :], in_=dout[0:64, :])
            nc.scalar.dma_start(out=csum[64:K, :], in_=dout[64:K, :])

            # ---- Sinkhorn ----
            # iter 1 row step, log space: L2 = csum - rowlse(csum)
            nM = sk.tile([K, 1], f32)
            nc.vector.reduce_max(
                out=nM, in_=csum, axis=mybir.AxisListType.X, negate=True
            )
            E = tps.tile([K, N], f32, tag="escr", bufs=1)
            S = sk.tile([K, 1], f32)
            nc.scalar.activation(out=E, in_=csum, func=Exp, bias=nM, accum_out=S)
            lS = sk.tile([K, 1], f32)
            nc.scalar.activation(out=lS, in_=S, func=Ln)
            L2 = sk.tile([K, N], f32)
            nc.vector.tensor_scalar(
                out=L2,
                in0=csum,
                scalar1=nM,
                scalar2=lS,
                op0=mybir.AluOpType.add,
                op1=mybir.AluOpType.subtract,
            )
            # iter 1 col step: log -> exp space.  e = exp(T - max) / sum
            TpP = tps.tile([N, K], f32)
            nc.tensor.transpose(TpP, L2, augk[:, 0:K])
            nM2 = sk.tile([N, 1], f32)
            nc.vector.reduce_max(
                out=nM2, in_=TpP, axis=mybir.AxisListType.X, negate=True
            )
            E2 = sk.tile([N, K], f32)
            S2 = sk.tile([N, 1], f32)
            nc.scalar.activation(out=E2, in_=TpP, func=Exp, bias=nM2, accum_out=S2)
            rS2 = sk.tile([N, 1], f32)
            nc.vector.reciprocal(out=rS2, in_=S2)
            e = sk.tile([N, K], f32)
            nc.vector.tensor_scalar(
                out=e, in0=E2, scalar1=rS2, scalar2=None,
                op0=mybir.AluOpType.mult,
            )

            # iters 2-5, exp space.  Invariant entering a row step: PSUM tile
            # Pk = [M^T | rowsums] with M [N,K]; entering a col step: PSUM
            # tile Pn = [M^T | colsums] with M [K,N].
            cur = e  # [N, K] in SBUF
            for it in range(1, ITERS):
                Pk = tps.tile([K, N + 1], f32, tag="pk", bufs=2)
                nc.tensor.matmul(Pk, cur, augn, start=True, stop=True)
                rR = sk.tile([K, 1], f32)
                nc.vector.reciprocal(out=rR, in_=Pk[:, N : N + 1])
                Mrow = sk.tile([K, N], f32)
                nc.vector.tensor_scalar(
                    out=Mrow, in0=Pk[:, 0:N], scalar1=rR,
                    scalar2=None, op0=mybir.AluOpType.mult,
                )
                Pn = tps.tile([N, K + 1], f32, tag="pn", bufs=2)
                nc.tensor.matmul(Pn, Mrow, augk, start=True, stop=True)
                rC = sk.tile([N, 1], f32)
                nc.vector.reciprocal(out=rC, in_=Pn[:, K : K + 1])
                if it < ITERS - 1:
                    Mcol = sk.tile([N, K], f32)
                    nc.vector.tensor_scalar(
                        out=Mcol, in0=Pn[:, 0:K], scalar1=rC,
                        scalar2=None, op0=mybir.AluOpType.mult,
                    )
                    cur = Mcol
                else:
                    res = sk.tile([N, K], f32)
                    nc.vector.tensor_scalar(
                        out=res[0:32, :], in0=Pn[0:32, 0:K],
                        scalar1=rC[0:32],
                        scalar2=None, op0=mybir.AluOpType.mult,
                    )
                    nc.sync.dma_start(out=out[0:32, :], in_=res[0:32, :])
                    nc.vector.tensor_scalar(
                        out=res[32:N, :], in0=Pn[32:N, 0:K],
                        scalar1=rC[32:N],
                        scalar2=None, op0=mybir.AluOpType.mult,
                    )
                    nc.scalar.dma_start(out=out[32:N, :], in_=res[32:N, :])

    nc.compile()
    return nc


def _get_nc():
    if "nc" not in _CACHE:
        _CACHE["nc"] = _build()
    return _CACHE["nc"]


def _get_runner():
    """Cached PJRT executable (mirrors bass2jax.run_bass_via_pjrt's multi-core
    branch) so repeat kernel() calls skip retracing/recompiling."""
    if "runner" in _CACHE:
        return _CACHE["runner"]
    import jax
    from jax.experimental.shard_map import shard_map
    from jax.sharding import Mesh, PartitionSpec

    from concourse import bass2jax, mybir

    nc = _get_nc()
    bass2jax.install_neuronx_cc_hook()
    assert nc.dbg_addr is None
    partition_name = nc.partition_id_tensor.name if nc.partition_id_tensor else None

    in_names, out_names, out_avals, out_shapes = [], [], [], []
    for alloc in nc.m.functions[0].allocations:
        if not isinstance(alloc, mybir.MemoryLocationSet):
            continue
        name = alloc.memorylocations[0].name
        if alloc.kind == "ExternalInput":
            if name != partition_name:
                in_names.append(name)
        elif alloc.kind == "ExternalOutput":
            shape = tuple(alloc.tensor_shape)
            dtype = mybir.dt.np(alloc.dtype)
            out_avals.append(jax.core.ShapedArray(shape, dtype))
            out_shapes.append((name, shape, dtype))
            out_names.append(name)
    n_params = len(in_names)
    n_outs = len(out_names)
    all_in_names = list(in_names) + list(out_names)
    if partition_name is not None:
        all_in_names.append(partition_name)
    donate = tuple(range(n_params, n_params + n_outs))

    def _body(*args):
        operands = list(args)
        if partition_name is not None:
            operands.append(bass2jax.partition_id_tensor())
        outs = bass2jax._bass_exec_p.bind(
            *operands,
            out_avals=tuple(out_avals),
            in_names=tuple(all_in_names),
            out_names=tuple(out_names),
            lowering_input_output_aliases=(),
            sim_require_finite=True,
            sim_require_nnan=True,
            nc=nc,
        )
        return tuple(outs)

    devices = jax.devices()[:CORES]
    mesh = Mesh(np.asarray(devices), ("core",))
    in_specs = (PartitionSpec("core"),) * (n_params + n_outs)
    out_specs = (PartitionSpec("core"),) * n_outs
    sharded = jax.jit(
        shard_map(
            _body, mesh=mesh, in_specs=in_specs, out_specs=out_specs, check_rep=False
        ),
        donate_argnums=donate,
        keep_unused=True,
    )
    _CACHE["runner"] = (sharded, in_names, out_shapes)
    return _CACHE["runner"]


def _pack(arr, rows):
    # arr [rows, HW] fp32 -> fp16 packed per core: q-step q of core c covers
    # HW [c*SHARD + q*512, +512), SBUF partition p holds FB=4 consecutive
    # elements per (q, row).  Blocks follow the BLOCKS ramp; all blocks of
    # one size are grouped into a single tensor.
    v = arr.reshape(rows, CORES, Q, P, FB).transpose(1, 2, 3, 0, 4)
    v = v.astype(np.float16)  # [CORES, Q, P, rows, FB]
    groups = {s: [] for s in SIZES}
    q0 = 0
    for s in BLOCKS:
        blk = v[:, q0 : q0 + s]  # [CORES, s, P, rows, FB]
        blk = blk.transpose(0, 2, 1, 3, 4).reshape(CORES, 1, P, s * rows * FB)
        groups[s].append(blk)
        q0 += s
    return {s: np.ascontiguousarray(np.concatenate(g, axis=1)) for s, g in groups.items()}


def kernel(pred_masks, gt_masks):
    global LAST_RESULT
    from concourse import bass_utils

    pred = np.ascontiguousarray(np.asarray(pred_masks, dtype=np.float32)).reshape(
        K, HW
    )
    gt = np.ascontiguousarray(np.asarray(gt_masks, dtype=np.float32)).reshape(N, HW)
    pk = _pack(pred, K)
    gk = _pack(gt, N)
    in_maps = [
        {
            **{f"p_{s}": pk[s][c] for s in SIZES},
            **{f"g_{s}": gk[s][c] for s in SIZES},
        }
        for c in range(CORES)
    ]
    if TRACE:
        nc = _get_nc()
        res = bass_utils.run_bass_kernel_spmd(
            nc, in_maps, core_ids=list(range(CORES)), trace=TRACE, **TRACE_KW
        )
        LAST_RESULT = res
        o = np.asarray(res.results[0]["out"], dtype=np.float32)
        return np.ascontiguousarray(o.T).reshape(1, K, N)

    sharded, in_names, out_shapes = _get_runner()
    concat_in = [
        np.concatenate([in_maps[c][name] for c in range(CORES)], axis=0)
        for name in in_names
    ]
    concat_zeros = [
        np.zeros((CORES * shape[0], *shape[1:]), dtype) for _, shape, dtype in out_shapes
    ]
    out_arrs = sharded(*concat_in, *concat_zeros)
    out0 = np.asarray(out_arrs[0]).reshape(CORES, N, K)[0]
    return np.ascontiguousarray(out0.astype(np.float32).T).reshape(1, K, N)


# revision 6
# speedup vs baseline: 1.3039x; 1.3039x over previous
"""DifferentiableMatcher Trainium2 kernel (v3).

cost[k, n] = 1 - <pred_k, gt_n> over HW=512*512, then 5 Sinkhorn iterations
(row/col normalizations) and exp.

Strategy (8 NeuronCores):
  - Shard the HW contraction: core c owns HW slice [c*32768, (c+1)*32768).
  - Inputs cast to fp16 on host (halves HBM traffic; fp32 PSUM accumulate).
  - Host packs each shard so SBUF partition p holds runs of FB=4 HW elements
    per (q, k); DMA per partition is fully contiguous.
  - Per core: 256 accumulating fp16 matmuls -> partial dot [100, 50] in PSUM.
    Block sizes ramp 1,1,2,4,8,...,8,4,2,1,1 q-steps so the PE starts on the
    first small block almost immediately and drains right as the DMA stream
    ends (PE period ~104ns/matmul ~= DMA rate).
  - AllReduce (20KB fp32) across the 8 cores.
  - Sinkhorn v3: iteration 1 runs in log space (row lse with max; col step
    computes exp(x - max)/sum directly), then iterations 2-5 run in EXP
    space: divide by row/col sums only.  Each transpose is a plain fp32
    matmul against [I | 1] which also emits the next step's sums column,
    so a half-step is just (tensor_scalar divide) + (matmul).  The final
    col division is the output (exp already applied by construction).

v3 changes vs v2:
  - DMA block ramp (PE drain overlapped with stream end).
  - Exp-space Sinkhorn: 2 instructions per half-step instead of 6.
"""

import numpy as np

K = 100
N = 50
HW = 512 * 512
CORES = 8
SHARD = HW // CORES  # 32768
P = 128
FB = 4
Q = SHARD // (P * FB)  # 64 q-steps per core
# q-steps per block: small ramp-in so the PE starts early, small ramp-out so
# the last matmuls aren't serialized behind one big DMA.
BLOCKS = [2, 4, 6, 8, 8, 8, 8, 8, 8, 2, 2]
assert sum(BLOCKS) == Q
SIZES = sorted(set(BLOCKS))  # [1, 2, 4, 8]
TEMP = 0.1
ITERS = 5

_CACHE = {}

TRACE = False
TRACE_KW = {}
LAST_RESULT = None


def _patch_act_tables():
    """Make the combined Exp+Ln table set the only candidate for Exp/Ln so
    the compiler emits one table load instead of thrashing per activation.
    Set positions (= act_func_set_ids) are preserved."""
    import concourse.hw_specs as hw_specs
    from concourse import bacc as bacc_mod
    from concourse import mybir

    if getattr(bacc_mod, "_act_tables_patched", False):
        return
    orig = hw_specs.get_activation_tables

    def patched(arch):
        t = orig(arch)
        exp = mybir.ActivationFunctionType.Exp
        ln = mybir.ActivationFunctionType.Ln
        out = {}
        for name, funcs in t.items():
            if (exp in funcs) != (ln in funcs):
                funcs = funcs - {exp, ln}
            out[name] = funcs
        return out

    bacc_mod.get_activation_tables = patched
    bacc_mod._act_tables_patched = True


def _build():
    from concourse import bacc, tile, mybir
    from concourse.masks import make_identity

    _patch_act_tables()

    f16 = mybir.dt.float16
    f32 = mybir.dt.float32
    nc = bacc.Bacc("TRN2", target_bir_lowering=False, debug=False, enable_asserts=False, num_devices=CORES, monotonic_sem_count=0, enable_partition_id=False)

    counts = {s: BLOCKS.count(s) for s in SIZES}
    p_in = {
        s: nc.dram_tensor(f"p_{s}", [counts[s], P, s * K * FB], f16, kind="ExternalInput").ap()
        for s in SIZES
    }
    g_in = {
        s: nc.dram_tensor(f"g_{s}", [counts[s], P, s * N * FB], f16, kind="ExternalInput").ap()
        for s in SIZES
    }
    out = nc.dram_tensor("out", [N, K], f32, kind="ExternalOutput").ap()

    Exp = mybir.ActivationFunctionType.Exp
    Ln = mybir.ActivationFunctionType.Ln

    with tile.TileContext(nc) as tc:
        with (
            tc.tile_pool(name="pp", bufs=len(BLOCKS)) as pp,
            tc.tile_pool(name="gp", bufs=len(BLOCKS)) as gp,
            tc.tile_pool(name="sk", bufs=1) as sk,
            tc.tile_pool(name="cps", bufs=1, space="PSUM") as cps,
            tc.tile_pool(name="tps", bufs=2, space="PSUM") as tps,
            tc.tile_pool(name="dram", bufs=1, space="DRAM") as dram,
        ):
            # Force the natural_log_exp table set to load now, while the
            # DMAs stream; the Sinkhorn tail then starts without the
            # 1.3us ACT_TABLE_LOAD on its critical path.
            warm = sk.tile([1, 1], f32)
            nc.vector.memset(warm, 0.0)
            nc.scalar.activation(out=warm, in_=warm, func=Exp)

            # [I_K | 1] and [I_N | 1]: transpose-with-sums right operands.
            # AUGK[:, 0:K] doubles as the identity for the iter-1 transpose.
            augk = sk.tile([K, K], f32)
            make_identity(nc, augk)
            augk16 = sk.tile([K, K + 1], f16)
            make_identity(nc, augk16[:, 0:K])
            nc.gpsimd.memset(augk16[:, K : K + 1], 1.0)
            augn16 = sk.tile([N, N + 1], f16)
            make_identity(nc, augn16[:, 0:N])
            nc.gpsimd.memset(augn16[:, N : N + 1], 1.0)

            C = cps.tile([K, N], f32)
            seen = {s: 0 for s in SIZES}
            nblk = len(BLOCKS)
            for bi, s in enumerate(BLOCKS):
                b = seen[s]
                seen[s] += 1
                PT = pp.tile([P, s * K * FB], f16)
                GT = gp.tile([P, s * N * FB], f16)
                if bi % 2 == 0:
                    nc.scalar.dma_start(out=GT, in_=g_in[s][b])
                    nc.sync.dma_start(out=PT, in_=p_in[s][b])
                else:
                    nc.sync.dma_start(out=GT, in_=g_in[s][b])
                    nc.scalar.dma_start(out=PT, in_=p_in[s][b])
                PT4 = PT.rearrange("p (q k f) -> p q k f", k=K, f=FB)
                GT4 = GT.rearrange("p (q n f) -> p q n f", n=N, f=FB)
                for q in range(s):
                    for f in range(FB):
                        nc.tensor.matmul(
                            C,
                            PT4[:, q, :, f],
                            GT4[:, q, :, f],
                            start=(bi == 0 and q == 0 and f == 0),
                            stop=(bi == nblk - 1 and q == s - 1 and f == FB - 1),
                        )

            # partial dot [K,N] -> (scale 1/TEMP) -> SBUF -> DRAM ->
            # AllReduce -> SBUF
            c_sb = sk.tile([K, N], f32)
            din = dram.tile([K, N], f32)
            dout = dram.tile([K, N], f32, addr_space="Shared")
            nc.vector.tensor_scalar(
                out=c_sb[0:64, :], in0=C[0:64, :], scalar1=1.0 / TEMP,
                scalar2=None, op0=mybir.AluOpType.mult,
            )
            nc.sync.dma_start(out=din[0:64, :], in_=c_sb[0:64, :])
            nc.vector.tensor_scalar(
                out=c_sb[64:K, :], in0=C[64:K, :], scalar1=1.0 / TEMP,
                scalar2=None, op0=mybir.AluOpType.mult,
            )
            nc.scalar.dma_start(out=din[64:K, :], in_=c_sb[64:K, :])
            nc.gpsimd.collective_compute(
                "AllReduce",
                mybir.AluOpType.add,
                replica_groups=[list(range(CORES))],
                ins=[din.opt()],
                outs=[dout.opt()],
            )
            csum = sk.tile([K, N], f32)
            nc.sync.dma_start(out=csum[0:64, :], in_=dout[0:64, :])
            nc.scalar.dma_start(out=csum[64:K, :], in_=dout[64:K, :])

            # ---- Sinkhorn ----
            # iter 1 row step, log space: L2 = csum - rowlse(csum)
            nM = sk.tile([K, 1], f32)
            nc.vector.reduce_max(
                out=nM, in_=csum, axis=mybir.AxisListType.X, negate=True
            )
            E = tps.tile([K, N], f32, tag="escr", bufs=1)
            S = sk.tile([K, 1], f32)
            nc.scalar.activation(out=E, in_=csum, func=Exp, bias=nM, accum_out=S)
            lS = sk.tile([K, 1], f32)
            nc.scalar.activation(out=lS, in_=S, func=Ln)
            L2 = sk.tile([K, N], f32)
            nc.vector.tensor_scalar(
                out=L2,
                in0=csum,
                scalar1=nM,
                scalar2=lS,
                op0=mybir.AluOpType.add,
                op1=mybir.AluOpType.subtract,
            )
            # iter 1 col step: log -> exp space.  e = exp(T - max) / sum
            TpP = tps.tile([N, K], f32)
            nc.tensor.transpose(TpP, L2, augk)
            nM2 = sk.tile([N, 1], f32)
            nc.vector.reduce_max(
                out=nM2, in_=TpP, axis=mybir.AxisListType.X, negate=True
            )
            E2 = sk.tile([N, K], f32)
            S2 = sk.tile([N, 1], f32)
            nc.scalar.activation(out=E2, in_=TpP, func=Exp, bias=nM2, accum_out=S2)
            rS2 = sk.tile([N, 1], f32)
            nc.vector.reciprocal(out=rS2, in_=S2)
            e = sk.tile([N, K], f16)
            nc.vector.tensor_scalar(
                out=e, in0=E2, scalar1=rS2, scalar2=None,
                op0=mybir.AluOpType.mult,
            )

            # iters 2-5, exp space.  Invariant entering a row step: PSUM tile
            # Pk = [M^T | rowsums] with M [N,K]; entering a col step: PSUM
            # tile Pn = [M^T | colsums] with M [K,N].
            cur = e  # [N, K] in SBUF
            for it in range(1, ITERS):
                Pk = tps.tile([K, N + 1], f32, tag="pk", bufs=2)
                nc.tensor.matmul(Pk, cur, augn16, start=True, stop=True)
                rR = sk.tile([K, 1], f32)
                nc.vector.reciprocal(out=rR, in_=Pk[:, N : N + 1])
                Mrow = sk.tile([K, N], f16)
                nc.vector.tensor_scalar(
                    out=Mrow, in0=Pk[:, 0:N], scalar1=rR,
                    scalar2=None, op0=mybir.AluOpType.mult,
                )
                Pn = tps.tile([N, K + 1], f32, tag="pn", bufs=2)
                nc.tensor.matmul(Pn, Mrow, augk16, start=True, stop=True)
                rC = sk.tile([N, 1], f32)
                nc.vector.reciprocal(out=rC, in_=Pn[:, K : K + 1])
                if it < ITERS - 1:
                    Mcol = sk.tile([N, K], f16)
                    nc.vector.tensor_scalar(
                        out=Mcol, in0=Pn[:, 0:K], scalar1=rC,
                        scalar2=None, op0=mybir.AluOpType.mult,
                    )
                    cur = Mcol
                else:
                    res = sk.tile([N, K], f32)
                    nc.vector.tensor_scalar(
                        out=res[0:32, :], in0=Pn[0:32, 0:K],
                        scalar1=rC[0:32],
                        scalar2=None, op0=mybir.AluOpType.mult,
                    )
                    nc.sync.dma_start(out=out[0:32, :], in_=res[0:32, :])
                    nc.vector.tensor_scalar(
                        out=res[32:N, :], in0=Pn[32:N, 0:K],
                        scalar1=rC[32:N],
                        scalar2=None, op0=mybir.AluOpType.mult,
                    )
                    nc.scalar.dma_start(out=out[32:N, :], in_=res[32:N, :])

    nc.compile()
    return nc


def _get_nc():
    if "nc" not in _CACHE:
        _CACHE["nc"] = _build()
    return _CACHE["nc"]


def _get_runner():
    """Cached PJRT executable (mirrors bass2jax.run_bass_via_pjrt's multi-core
    branch) so repeat kernel() calls skip retracing/recompiling."""
    if "runner" in _CACHE:
        return _CACHE["runner"]
    import jax
    from jax.experimental.shard_map import shard_map
    from jax.sharding import Mesh, PartitionSpec

    from concourse import bass2jax, mybir

    nc = _get_nc()
    bass2jax.install_neuronx_cc_hook()
    assert nc.dbg_addr is None
    partition_name = nc.partition_id_tensor.name if nc.partition_id_tensor else None

    in_names, out_names, out_avals, out_shapes = [], [], [], []
    for alloc in nc.m.functions[0].allocations:
        if not isinstance(alloc, mybir.MemoryLocationSet):
            continue
        name = alloc.memorylocations[0].name
        if alloc.kind == "ExternalInput":
            if name != partition_name:
                in_names.append(name)
        elif alloc.kind == "ExternalOutput":
            shape = tuple(alloc.tensor_shape)
            dtype = mybir.dt.np(alloc.dtype)
            out_avals.append(jax.core.ShapedArray(shape, dtype))
            out_shapes.append((name, shape, dtype))
            out_names.append(name)
    n_params = len(in_names)
    n_outs = len(out_names)
    all_in_names = list(in_names) + list(out_names)
    if partition_name is not None:
        all_in_names.append(partition_name)
    donate = tuple(range(n_params, n_params + n_outs))

    def _body(*args):
        operands = list(args)
        if partition_name is not None:
            operands.append(bass2jax.partition_id_tensor())
        outs = bass2jax._bass_exec_p.bind(
            *operands,
            out_avals=tuple(out_avals),
            in_names=tuple(all_in_names),
            out_names=tuple(out_names),
            lowering_input_output_aliases=(),
            sim_require_finite=True,
            sim_require_nnan=True,
            nc=nc,
        )
        return tuple(outs)

    devices = jax.devices()[:CORES]
    mesh = Mesh(np.asarray(devices), ("core",))
    in_specs = (PartitionSpec("core"),) * (n_params + n_outs)
    out_specs = (PartitionSpec("core"),) * n_outs
    sharded = jax.jit(
        shard_map(
            _body, mesh=mesh, in_specs=in_specs, out_specs=out_specs, check_rep=False
        ),
        donate_argnums=donate,
        keep_unused=True,
    )
    _CACHE["runner"] = (sharded, in_names, out_shapes)
    return _CACHE["runner"]


def _pack(arr, rows):
    # arr [rows, HW] fp32 -> fp16 packed per core: q-step q of core c covers
    # HW [c*SHARD + q*512, +512), SBUF partition p holds FB=4 consecutive
    # elements per (q, row).  Blocks follow the BLOCKS ramp; all blocks of
    # one size are grouped into a single tensor.
    v = arr.reshape(rows, CORES, Q, P, FB).transpose(1, 2, 3, 0, 4)
    v = v.astype(np.float16)  # [CORES, Q, P, rows, FB]
    groups = {s: [] for s in SIZES}
    q0 = 0
    for s in BLOCKS:
        blk = v[:, q0 : q0 + s]  # [CORES, s, P, rows, FB]
        blk = blk.transpose(0, 2, 1, 3, 4).reshape(CORES, 1, P, s * rows * FB)
        groups[s].append(blk)
        q0 += s
    return {s: np.ascontiguousarray(np.concatenate(g, axis=1)) for s, g in groups.items()}


def kernel(pred_masks, gt_masks):
    global LAST_RESULT
    from concourse import bass_utils

    pred = np.ascontiguousarray(np.asarray(pred_masks, dtype=np.float32)).reshape(
        K, HW
    )
    gt = np.ascontiguousarray(np.asarray(gt_masks, dtype=np.float32)).reshape(N, HW)
    pk = _pack(pred, K)
    gk = _pack(gt, N)
    in_maps = [
        {
            **{f"p_{s}": pk[s][c] for s in SIZES},
            **{f"g_{s}": gk[s][c] for s in SIZES},
        }
        for c in range(CORES)
    ]
    if TRACE:
        nc = _get_nc()
        res = bass_utils.run_bass_kernel_spmd(
            nc, in_maps, core_ids=list(range(CORES)), trace=TRACE, **TRACE_KW
        )
        LAST_RESULT = res
        o = np.asarray(res.results[0]["out"], dtype=np.float32)
        return np.ascontiguousarray(o.T).reshape(1, K, N)

    sharded, in_names, out_shapes = _get_runner()
    concat_in = [
        np.concatenate([in_maps[c][name] for c in range(CORES)], axis=0)
        for name in in_names
    ]
    concat_zeros = [
        np.zeros((CORES * shape[0], *shape[1:]), dtype) for _, shape, dtype in out_shapes
    ]
    out_arrs = sharded(*concat_in, *concat_zeros)
    out0 = np.asarray(out_arrs[0]).reshape(CORES, N, K)[0]
    return np.ascontiguousarray(out0.astype(np.float32).T).reshape(1, K, N)
